# revision 3
# baseline (speedup 1.0000x reference)
"""Trainium2 Bass kernel for DeepDFT pairwise expansion.

Computes, for source probes s[N=4096,3] and target atoms t[M=2048,3]:
  dist[n,m]      = |t[m] - s[n]|
  direction      = (t[m]-s[n]) / (dist + 1e-11)
  mask           = dist < 4.0
  expansion[...] = sin(k_n * dist) / (BOHR*dist + 1e-11),  k_n = n*pi*BOHR/4

Sharding: source axis N split across 8 NeuronCores (512 rows each);
targets replicated. Each core emits 25 f32 [512,2048] planes + u8 mask.

Per [128,512] block:
  PE   : diff_c = t_c - s_c  (K=2 matmul: lhsT=[ones; s_c], rhs=[t_c; -1])
  ACT  : squares of diffs (from PSUM), sqrt, 20x sin (range-reduced args)
  POOL : dist^2 adds + a slice of the expansion multiplies
  DVE  : mask (dist^2<16), d, 1/d, direction, magic-round range reduction
Range reduction: q = round(arg/2pi) via float32 magic-number add;
w = dist - q*(2pi/k_n); sin via ACT with scale=k_n (arg lands in [-pi,pi]).
"""
from contextlib import ExitStack

import numpy as np

N_FULL = 4096
M_FULL = 2048
FEAT = 20
N_CORES = 8
N_LOC = N_FULL // N_CORES          # 512
P = 128                            # SBUF partitions
N_TILES = N_LOC // P               # 4 row-tiles per core
M_CHUNK = 512
M_CHUNKS = M_FULL // M_CHUNK       # 4

CUT = 4.0
EPS = 1e-11
BOHR = 0.5291772105638411
TWO_PI = 2.0 * np.pi
MAGIC = float(1.5 * 2.0**23)

K_N = [float(n * np.pi * BOHR / CUT) for n in range(1, FEAT + 1)]
# feature indices whose final multiply runs on GpSimd (rest on DVE)
POOL_NS = set(range(8, FEAT))

_RUNTIME = {}


def _build_program():
    import concourse.tile as tile
    import concourse.mybir as mybir
    from concourse import bacc

    nc = bacc.Bacc("TRN2", target_bir_lowering=False, debug=False)
    f32 = mybir.dt.float32

    tr_d = [nc.dram_tensor(f"tr{c}", [2, M_FULL], f32, kind="ExternalInput")
            for c in range(3)]
    sl_d = [nc.dram_tensor(f"sl{c}", [2, N_LOC], f32, kind="ExternalInput")
            for c in range(3)]

    dist_d = nc.dram_tensor("dist_o", [N_LOC, M_FULL], f32, kind="ExternalOutput")
    mask_d = nc.dram_tensor("mask_o", [N_LOC, M_FULL], mybir.dt.uint8,
                            kind="ExternalOutput")
    dir_d = [nc.dram_tensor(f"dir{c}_o", [N_LOC, M_FULL], f32, kind="ExternalOutput")
             for c in range(3)]
    exp_d = [nc.dram_tensor(f"exp{n}_o", [N_LOC, M_FULL], f32, kind="ExternalOutput")
             for n in range(FEAT)]

    with tile.TileContext(nc) as tc, ExitStack() as ctx:
        const = ctx.enter_context(tc.tile_pool(name="const", bufs=1))
        sb = ctx.enter_context(tc.tile_pool(name="sb", bufs=2))
        ps = ctx.enter_context(tc.tile_pool(name="ps", bufs=6, space="PSUM"))

        tr = [const.tile([2, M_FULL], f32, name=f"tr{c}_t") for c in range(3)]
        sl = [const.tile([2, N_LOC], f32, name=f"sl{c}_t") for c in range(3)]
        for c in range(3):
            nc.sync.dma_start(tr[c][:], tr_d[c].ap())
            nc.sync.dma_start(sl[c][:], sl_d[c].ap())

        for r in range(N_TILES):
            rows = slice(r * P, (r + 1) * P)
            for j in range(M_CHUNKS):
                cols = slice(j * M_CHUNK, (j + 1) * M_CHUNK)

                diff = []
                for c in range(3):
                    dps = ps.tile([P, M_CHUNK], f32, tag="diff")
                    nc.tensor.matmul(dps[:], sl[c][:, rows], tr[c][:, cols],
                                     start=True, stop=True)
                    diff.append(dps)

                sq = []
                for c in range(3):
                    s = sb.tile([P, M_CHUNK], f32, tag="sq")
                    nc.scalar.square(s[:], diff[c][:])
                    sq.append(s)

                s01 = sb.tile([P, M_CHUNK], f32, tag="s01")
                nc.gpsimd.tensor_tensor(s01[:], sq[0][:], sq[1][:],
                                        mybir.AluOpType.add)
                dist2 = sb.tile([P, M_CHUNK], f32, tag="dist2")
                nc.gpsimd.tensor_tensor(dist2[:], s01[:], sq[2][:],
                                        mybir.AluOpType.add)

                mask_t = sb.tile([P, M_CHUNK], mybir.dt.uint8, tag="mask")
                nc.vector.tensor_scalar(mask_t[:], dist2[:], float(CUT * CUT), None,
                                        mybir.AluOpType.is_lt)
                nc.sync.dma_start(mask_d.ap()[rows, cols], mask_t[:])

                dist_t = sb.tile([P, M_CHUNK], f32, tag="dist")
                nc.scalar.sqrt(dist_t[:], dist2[:])
                nc.sync.dma_start(dist_d.ap()[rows, cols], dist_t[:])

                d_t = sb.tile([P, M_CHUNK], f32, tag="d")
                nc.vector.tensor_scalar(d_t[:], dist_t[:], BOHR, EPS,
                                        mybir.AluOpType.mult, mybir.AluOpType.add)
                recip = sb.tile([P, M_CHUNK], f32, tag="recip")
                nc.vector.reciprocal(recip[:], d_t[:])

                for c in range(3):
                    dir_t = sb.tile([P, M_CHUNK], f32, tag="dir")
                    nc.vector.scalar_tensor_tensor(dir_t[:], diff[c][:], BOHR,
                                                   recip[:], mybir.AluOpType.mult,
                                                   mybir.AluOpType.mult)
                    nc.sync.dma_start(dir_d[c].ap()[rows, cols], dir_t[:])

                for n in range(FEAT):
                    kn = K_N[n]
                    q = sb.tile([P, M_CHUNK], mybir.dt.int32, tag="q", bufs=3)
                    nc.vector.tensor_scalar(q[:], dist_t[:], kn / TWO_PI, None,
                                            mybir.AluOpType.mult)
                    w = sb.tile([P, M_CHUNK], f32, tag="w", bufs=3)
                    nc.vector.scalar_tensor_tensor(w[:], q[:], -TWO_PI / kn,
                                                   dist_t[:], mybir.AluOpType.mult,
                                                   mybir.AluOpType.add)
                    sin_t = sb.tile([P, M_CHUNK], f32, tag="sin", bufs=4)
                    nc.scalar.activation(sin_t[:], w[:],
                                         mybir.ActivationFunctionType.Sin,
                                         scale=kn)
                    exp_t = sb.tile([P, M_CHUNK], f32, tag="exp", bufs=6)
                    if n in POOL_NS:
                        nc.gpsimd.tensor_tensor(exp_t[:], sin_t[:], recip[:],
                                                mybir.AluOpType.mult)
                    else:
                        nc.vector.tensor_tensor(exp_t[:], sin_t[:], recip[:],
                                                mybir.AluOpType.mult)
                    nc.sync.dma_start(exp_d[n].ap()[rows, cols], exp_t[:])

    nc.compile()
    return nc


def _get_nc():
    if "nc" not in _RUNTIME:
        _RUNTIME["nc"] = _build_program()
    return _RUNTIME["nc"]


def make_in_maps(source, target):
    """Host-side packing: per-core input dicts."""
    source = np.asarray(source, np.float32).reshape(N_FULL, 3)
    target = np.asarray(target, np.float32).reshape(M_FULL, 3)
    trs = {}
    for c in range(3):
        t2 = np.empty((2, M_FULL), np.float32)
        t2[0] = target[:, c]
        t2[1] = -1.0
        trs[f"tr{c}"] = t2
    in_maps = []
    for core in range(N_CORES):
        s = source[core * N_LOC:(core + 1) * N_LOC]
        m = dict(trs)
        for c in range(3):
            s2 = np.empty((2, N_LOC), np.float32)
            s2[0] = 1.0
            s2[1] = s[:, c]
            m[f"sl{c}"] = s2
        in_maps.append(m)
    return in_maps


def assemble(results):
    """Gather per-core output dicts into the full reference-shaped tuple."""
    dist = np.concatenate([r["dist_o"] for r in results], 0)[None]
    mask = np.concatenate([r["mask_o"] for r in results], 0)[None].astype(bool)
    direction = np.stack(
        [np.concatenate([r[f"dir{c}_o"] for r in results], 0) for c in range(3)],
        axis=-1)[None]
    expansion = np.stack(
        [np.concatenate([r[f"exp{n}_o"] for r in results], 0) for n in range(FEAT)],
        axis=-1)[None]
    return dist, direction, mask, expansion


def kernel(source, target):
    from concourse import bass2jax

    nc = _get_nc()
    in_maps = make_in_maps(source, target)
    results = bass2jax.run_bass_via_pjrt(nc, in_maps, n_cores=N_CORES)
    return assemble(results)


# revision 7
# speedup vs baseline: 1.1789x; 1.1789x over previous
"""Trainium2 Bass kernel for DeepDFT pairwise expansion.

Computes, for source probes s[N=4096,3] and target atoms t[M=2048,3]:
  dist[n,m]      = |t[m] - s[n]|
  direction      = (t[m]-s[n]) / (dist + 1e-11)
  mask           = dist < 4.0
  expansion[...] = sin(k_n * dist) / (BOHR*dist + 1e-11),  k_n = n*pi*BOHR/4

Sharding: source axis N split across 8 NeuronCores (512 rows each);
targets replicated. Each core emits 25 f32 [512,2048] planes + u8 mask.

Per [128,512] block:
  PE   : diff_c = t_c - s_c  (K=2 matmul: lhsT=[ones; s_c], rhs=[t_c; -1])
  ACT  : squares of diffs (from PSUM), sqrt, 20x sin (range-reduced args)
  POOL : dist^2 adds + a slice of the expansion multiplies
  DVE  : mask (dist^2<16), d, 1/d, direction, magic-round range reduction
Range reduction: q = round(arg/2pi) via float32 magic-number add;
w = dist - q*(2pi/k_n); sin via ACT with scale=k_n (arg lands in [-pi,pi]).
"""
from contextlib import ExitStack

import numpy as np

N_FULL = 4096
M_FULL = 2048
FEAT = 20
N_CORES = 8
N_LOC = N_FULL // N_CORES          # 512
P = 128                            # SBUF partitions
N_TILES = N_LOC // P               # 4 row-tiles per core
M_CHUNK = 1024
M_CHUNKS = M_FULL // M_CHUNK

CUT = 4.0
EPS = 1e-11
BOHR = 0.5291772105638411
TWO_PI = 2.0 * np.pi
MAGIC = float(1.5 * 2.0**23)

K_N = [float(n * np.pi * BOHR / CUT) for n in range(1, FEAT + 1)]
# feature indices whose final multiply runs on GpSimd (rest on DVE)
POOL_NS = set(range(8, FEAT))

_RUNTIME = {}
REPS = 1  # body repetitions (measurement only; overwrites same outputs)


def _raw_activation(eng, out_ap, in_ap, func, scale=1.0, bias=0.0):
    import concourse.mybir as mybir
    ins = [eng.lower_ap(in_ap),
           mybir.ImmediateValue(dtype=mybir.dt.float32, value=float(bias)),
           mybir.ImmediateValue(dtype=mybir.dt.float32, value=float(scale)),
           mybir.ImmediateValue(dtype=mybir.dt.float32, value=0.0)]
    return eng.add_instruction(mybir.InstActivation(
        name=eng.bass.get_next_instruction_name(), func=func,
        ins=ins, outs=[eng.lower_ap(out_ap)]))


def _build_program():
    import concourse.tile as tile
    import concourse.mybir as mybir
    from concourse import bacc

    nc = bacc.Bacc("TRN2", target_bir_lowering=False, debug=False)
    f32 = mybir.dt.float32

    tr_d = [nc.dram_tensor(f"tr{c}", [2, M_FULL], f32, kind="ExternalInput")
            for c in range(3)]
    sl_d = [nc.dram_tensor(f"sl{c}", [2, N_LOC], f32, kind="ExternalInput")
            for c in range(3)]

    dist_d = nc.dram_tensor("dist_o", [N_LOC, M_FULL], f32, kind="ExternalOutput")
    mask_d = nc.dram_tensor("mask_o", [N_LOC, M_FULL], mybir.dt.uint8,
                            kind="ExternalOutput")
    dir_d = [nc.dram_tensor(f"dir{c}_o", [N_LOC, M_FULL], f32, kind="ExternalOutput")
             for c in range(3)]
    exp_d = [nc.dram_tensor(f"exp{n}_o", [N_LOC, M_FULL], f32, kind="ExternalOutput")
             for n in range(FEAT)]

    with tile.TileContext(nc) as tc, ExitStack() as ctx:
        const = ctx.enter_context(tc.tile_pool(name="const", bufs=1))
        sb = ctx.enter_context(tc.tile_pool(name="sb", bufs=2))
        ps = ctx.enter_context(tc.tile_pool(name="ps", bufs=3, space="PSUM"))

        tr = [const.tile([2, M_FULL], f32, name=f"tr{c}_t") for c in range(3)]
        sl = [const.tile([2, N_LOC], f32, name=f"sl{c}_t") for c in range(3)]
        for c in range(3):
            nc.sync.dma_start(tr[c][:], tr_d[c].ap())
            nc.sync.dma_start(sl[c][:], sl_d[c].ap())

        for _rep in range(REPS):
          for r in range(N_TILES):
            rows = slice(r * P, (r + 1) * P)
            for j in range(M_CHUNKS):
                cols = slice(j * M_CHUNK, (j + 1) * M_CHUNK)

                diff = []
                for c in range(3):
                    dps = ps.tile([P, M_CHUNK], f32, tag="diff")
                    for h in range(M_CHUNK // 512):
                        nc.tensor.matmul(
                            dps[:, h * 512:(h + 1) * 512], sl[c][:, rows],
                            tr[c][:, j * M_CHUNK + h * 512:
                                   j * M_CHUNK + (h + 1) * 512],
                            start=True, stop=True)
                    diff.append(dps)

                sq = []
                for c in range(3):
                    s = sb.tile([P, M_CHUNK], f32, tag="sq")
                    nc.scalar.square(s[:], diff[c][:])
                    sq.append(s)

                s01 = sb.tile([P, M_CHUNK], f32, tag="s01")
                nc.gpsimd.tensor_tensor(s01[:], sq[0][:], sq[1][:],
                                        mybir.AluOpType.add)
                dist2 = sb.tile([P, M_CHUNK], f32, tag="dist2")
                nc.gpsimd.tensor_tensor(dist2[:], s01[:], sq[2][:],
                                        mybir.AluOpType.add)

                mask_t = sb.tile([P, M_CHUNK], mybir.dt.uint8, tag="mask")
                nc.vector.tensor_scalar(mask_t[:], dist2[:], float(CUT * CUT), None,
                                        mybir.AluOpType.is_lt)
                nc.sync.dma_start(mask_d.ap()[rows, cols], mask_t[:])

                dist_t = sb.tile([P, M_CHUNK], f32, tag="dist")
                nc.scalar.sqrt(dist_t[:], dist2[:])
                nc.sync.dma_start(dist_d.ap()[rows, cols], dist_t[:])

                r0 = sb.tile([P, M_CHUNK], f32, tag="r0")
                _raw_activation(nc.scalar, r0[:], dist_t[:],
                                mybir.ActivationFunctionType.Reciprocal,
                                scale=BOHR, bias=EPS)
                nt = sb.tile([P, M_CHUNK], f32, tag="nt")
                nc.vector.scalar_tensor_tensor(nt[:], dist_t[:], BOHR, r0[:],
                                               mybir.AluOpType.mult,
                                               mybir.AluOpType.mult)
                nu = sb.tile([P, M_CHUNK], f32, tag="nu")
                nc.vector.tensor_scalar(nu[:], nt[:], -1.0, 2.0,
                                        mybir.AluOpType.mult, mybir.AluOpType.add)
                recip = sb.tile([P, M_CHUNK], f32, tag="recip")
                nc.vector.tensor_tensor(recip[:], nu[:], r0[:],
                                        mybir.AluOpType.mult)

                for c in range(3):
                    dir_t = sb.tile([P, M_CHUNK], f32, tag="dir")
                    nc.vector.scalar_tensor_tensor(dir_t[:], diff[c][:], BOHR,
                                                   recip[:], mybir.AluOpType.mult,
                                                   mybir.AluOpType.mult)
                    nc.sync.dma_start(dir_d[c].ap()[rows, cols], dir_t[:])

                for n in range(FEAT):
                    kn = K_N[n]
                    q = sb.tile([P, M_CHUNK], mybir.dt.int32, tag="q", bufs=3)
                    nc.vector.tensor_scalar(q[:], dist_t[:], kn / TWO_PI, None,
                                            mybir.AluOpType.mult)
                    w = sb.tile([P, M_CHUNK], f32, tag="w", bufs=3)
                    nc.vector.scalar_tensor_tensor(w[:], q[:], -TWO_PI / kn,
                                                   dist_t[:], mybir.AluOpType.mult,
                                                   mybir.AluOpType.add)
                    sin_t = sb.tile([P, M_CHUNK], f32, tag="sin", bufs=4)
                    nc.scalar.activation(sin_t[:], w[:],
                                         mybir.ActivationFunctionType.Sin,
                                         scale=kn)
                    exp_t = sb.tile([P, M_CHUNK], f32, tag="exp", bufs=6)
                    if n in POOL_NS:
                        nc.gpsimd.tensor_tensor(exp_t[:], sin_t[:], recip[:],
                                                mybir.AluOpType.mult)
                    else:
                        nc.vector.tensor_tensor(exp_t[:], sin_t[:], recip[:],
                                                mybir.AluOpType.mult)
                    nc.sync.dma_start(exp_d[n].ap()[rows, cols], exp_t[:])

    nc.compile()
    return nc


def _get_nc():
    if "nc" not in _RUNTIME:
        _RUNTIME["nc"] = _build_program()
    return _RUNTIME["nc"]


def make_in_maps(source, target):
    """Host-side packing: per-core input dicts."""
    source = np.asarray(source, np.float32).reshape(N_FULL, 3)
    target = np.asarray(target, np.float32).reshape(M_FULL, 3)
    trs = {}
    for c in range(3):
        t2 = np.empty((2, M_FULL), np.float32)
        t2[0] = target[:, c]
        t2[1] = -1.0
        trs[f"tr{c}"] = t2
    in_maps = []
    for core in range(N_CORES):
        s = source[core * N_LOC:(core + 1) * N_LOC]
        m = dict(trs)
        for c in range(3):
            s2 = np.empty((2, N_LOC), np.float32)
            s2[0] = 1.0
            s2[1] = s[:, c]
            m[f"sl{c}"] = s2
        in_maps.append(m)
    return in_maps


def assemble(results):
    """Gather per-core output dicts into the full reference-shaped tuple."""
    dist = np.concatenate([r["dist_o"] for r in results], 0)[None]
    mask = np.concatenate([r["mask_o"] for r in results], 0)[None].astype(bool)
    direction = np.stack(
        [np.concatenate([r[f"dir{c}_o"] for r in results], 0) for c in range(3)],
        axis=-1)[None]
    expansion = np.stack(
        [np.concatenate([r[f"exp{n}_o"] for r in results], 0) for n in range(FEAT)],
        axis=-1)[None]
    return dist, direction, mask, expansion


def kernel(source, target):
    from concourse import bass2jax

    nc = _get_nc()
    in_maps = make_in_maps(source, target)
    results = bass2jax.run_bass_via_pjrt(nc, in_maps, n_cores=N_CORES)
    return assemble(results)


# revision 15
# speedup vs baseline: 133.7376x; 113.4414x over previous
"""Trainium2 Bass kernel for DeepDFT pairwise expansion.

Computes, for source probes s[N=4096,3] and target atoms t[M=2048,3]:
  dist[n,m]      = |t[m] - s[n]|
  direction      = (t[m]-s[n]) / (dist + 1e-11)
  mask           = dist < 4.0
  expansion[...] = sin(k_n * dist) / (BOHR*dist + 1e-11),  k_n = n*pi*BOHR/4

Sharding: source axis N split across 8 NeuronCores (512 rows each);
targets replicated. Each core emits 25 f32 [512,2048] planes + u8 mask.

Per [128,512] block:
  PE   : diff_c = t_c - s_c  (K=2 matmul: lhsT=[ones; s_c], rhs=[t_c; -1])
  ACT  : squares of diffs (from PSUM), sqrt, 20x sin (range-reduced args)
  POOL : dist^2 adds + a slice of the expansion multiplies
  DVE  : mask (dist^2<16), d, 1/d, direction, magic-round range reduction
Range reduction: q = round(arg/2pi) via float32 magic-number add;
w = dist - q*(2pi/k_n); sin via ACT with scale=k_n (arg lands in [-pi,pi]).
"""
from contextlib import ExitStack

import numpy as np

N_FULL = 4096
M_FULL = 2048
FEAT = 20
N_CORES = 8
N_LOC = N_FULL // N_CORES          # 512
P = 128                            # SBUF partitions
N_TILES = N_LOC // P               # 4 row-tiles per core
M_CHUNK = 1024
M_CHUNKS = M_FULL // M_CHUNK

CUT = 4.0
EPS = 1e-11
BOHR = 0.5291772105638411
TWO_PI = 2.0 * np.pi
MAGIC = float(1.5 * 2.0**23)

K_N = [float(n * np.pi * BOHR / CUT) for n in range(1, FEAT + 1)]
# feature indices whose final multiply runs on GpSimd (rest on DVE)
POOL_NS = set(range(FEAT))

_RUNTIME = {}
REPS = 1        # body repetitions (measurement only; overwrites same outputs)
RECIP_MODE = "newton"   # "newton" (ACT recip + 1 NR step) or "dve" (8-cyc divide)
PS_BUFS = 3             # PSUM diff tile slots
Q_ACT_NS = {0, 1, 2, 3}  # features whose round() runs on ACT (Identity -> i32)
SQ_DVE = False          # squares on DVE instead of ACT (model probe)
MASK_F32 = False        # mask as f32 plane (model probe: +3% DMA)
W_POOL_NS = set()       # features whose round+w run on GpSimd via magic add


def _raw_activation(eng, out_ap, in_ap, func, scale=1.0, bias=0.0):
    import concourse.mybir as mybir
    ins = [eng.lower_ap(in_ap),
           mybir.ImmediateValue(dtype=mybir.dt.float32, value=float(bias)),
           mybir.ImmediateValue(dtype=mybir.dt.float32, value=float(scale)),
           mybir.ImmediateValue(dtype=mybir.dt.float32, value=0.0)]
    return eng.add_instruction(mybir.InstActivation(
        name=eng.bass.get_next_instruction_name(), func=func,
        ins=ins, outs=[eng.lower_ap(out_ap)]))


def _build_program():
    import concourse.tile as tile
    import concourse.mybir as mybir
    from concourse import bacc

    global M_CHUNKS
    M_CHUNKS = M_FULL // M_CHUNK

    nc = bacc.Bacc("TRN2", target_bir_lowering=False, debug=False)
    f32 = mybir.dt.float32

    tr_d = [nc.dram_tensor(f"tr{c}", [2, M_FULL], f32, kind="ExternalInput")
            for c in range(3)]
    sl_d = [nc.dram_tensor(f"sl{c}", [2, N_LOC], f32, kind="ExternalInput")
            for c in range(3)]

    dist_d = nc.dram_tensor("dist_o", [N_LOC, M_FULL], f32, kind="ExternalOutput")
    mask_d = nc.dram_tensor("mask_o", [N_LOC, M_FULL],
                            f32 if MASK_F32 else mybir.dt.uint8,
                            kind="ExternalOutput")
    dir_d = [nc.dram_tensor(f"dir{c}_o", [N_LOC, M_FULL], f32, kind="ExternalOutput")
             for c in range(3)]
    exp_d = [nc.dram_tensor(f"exp{n}_o", [N_LOC, M_FULL], f32, kind="ExternalOutput")
             for n in range(FEAT)]

    with tile.TileContext(nc) as tc, ExitStack() as ctx:
        const = ctx.enter_context(tc.tile_pool(name="const", bufs=1))
        sb = ctx.enter_context(tc.tile_pool(name="sb", bufs=2))
        ps = ctx.enter_context(tc.tile_pool(name="ps", bufs=PS_BUFS, space="PSUM"))

        tr = [const.tile([2, M_FULL], f32, name=f"tr{c}_t") for c in range(3)]
        sl = [const.tile([2, N_LOC], f32, name=f"sl{c}_t") for c in range(3)]
        for c in range(3):
            nc.sync.dma_start(tr[c][:], tr_d[c].ap())
            nc.sync.dma_start(sl[c][:], sl_d[c].ap())
        magic_t = None
        if W_POOL_NS:
            magic_t = const.tile([P, M_CHUNK], f32, name="magic_t")
            nc.vector.memset(magic_t[:], MAGIC)

        for _rep in range(REPS):
          for r in range(N_TILES):
            rows = slice(r * P, (r + 1) * P)
            for j in range(M_CHUNKS):
                cols = slice(j * M_CHUNK, (j + 1) * M_CHUNK)

                diff = []
                for c in range(3):
                    dps = ps.tile([P, M_CHUNK], f32, tag="diff")
                    for h in range(M_CHUNK // 512):
                        nc.tensor.matmul(
                            dps[:, h * 512:(h + 1) * 512], sl[c][:, rows],
                            tr[c][:, j * M_CHUNK + h * 512:
                                   j * M_CHUNK + (h + 1) * 512],
                            start=True, stop=True)
                    diff.append(dps)

                sq = []
                for c in range(3):
                    s = sb.tile([P, M_CHUNK], f32, tag="sq", bufs=4)
                    if SQ_DVE:
                        nc.vector.tensor_tensor(s[:], diff[c][:], diff[c][:],
                                                mybir.AluOpType.mult)
                    else:
                        nc.scalar.square(s[:], diff[c][:])
                    sq.append(s)

                s01 = sb.tile([P, M_CHUNK], f32, tag="s01")
                nc.gpsimd.tensor_tensor(s01[:], sq[0][:], sq[1][:],
                                        mybir.AluOpType.add)
                dist2 = sb.tile([P, M_CHUNK], f32, tag="dist2")
                nc.gpsimd.tensor_tensor(dist2[:], s01[:], sq[2][:],
                                        mybir.AluOpType.add)

                mask_t = sb.tile([P, M_CHUNK],
                                 f32 if MASK_F32 else mybir.dt.uint8, tag="mask", bufs=3)
                nc.vector.tensor_scalar(mask_t[:], dist2[:], float(CUT * CUT), None,
                                        mybir.AluOpType.is_lt)
                nc.sync.dma_start(mask_d.ap()[rows, cols], mask_t[:])

                dist_t = sb.tile([P, M_CHUNK], f32, tag="dist")
                nc.scalar.sqrt(dist_t[:], dist2[:])
                nc.sync.dma_start(dist_d.ap()[rows, cols], dist_t[:])

                if RECIP_MODE == "act":
                    recip = sb.tile([P, M_CHUNK], f32, tag="recip")
                    _raw_activation(nc.scalar, recip[:], dist_t[:],
                                    mybir.ActivationFunctionType.Reciprocal,
                                    scale=BOHR, bias=EPS)
                elif RECIP_MODE == "newton":
                    r0 = sb.tile([P, M_CHUNK], f32, tag="r0")
                    _raw_activation(nc.scalar, r0[:], dist_t[:],
                                    mybir.ActivationFunctionType.Reciprocal,
                                    scale=BOHR, bias=EPS)
                    nt = sb.tile([P, M_CHUNK], f32, tag="nt")
                    nc.vector.scalar_tensor_tensor(nt[:], dist_t[:], BOHR, r0[:],
                                                   mybir.AluOpType.mult,
                                                   mybir.AluOpType.mult)
                    nu = sb.tile([P, M_CHUNK], f32, tag="nu")
                    nc.vector.tensor_scalar(nu[:], nt[:], -1.0, 2.0,
                                            mybir.AluOpType.mult,
                                            mybir.AluOpType.add)
                    recip = sb.tile([P, M_CHUNK], f32, tag="recip")
                    nc.vector.tensor_tensor(recip[:], nu[:], r0[:],
                                            mybir.AluOpType.mult)
                else:
                    d_t = sb.tile([P, M_CHUNK], f32, tag="d")
                    nc.vector.tensor_scalar(d_t[:], dist_t[:], BOHR, EPS,
                                            mybir.AluOpType.mult,
                                            mybir.AluOpType.add)
                    recip = sb.tile([P, M_CHUNK], f32, tag="recip")
                    nc.vector.reciprocal(recip[:], d_t[:])

                for c in range(3):
                    dir_t = sb.tile([P, M_CHUNK], f32, tag="dir", bufs=6)
                    nc.vector.scalar_tensor_tensor(dir_t[:], diff[c][:], BOHR,
                                                   recip[:], mybir.AluOpType.mult,
                                                   mybir.AluOpType.mult)
                    nc.sync.dma_start(dir_d[c].ap()[rows, cols], dir_t[:])

                for n in range(FEAT):
                    kn = K_N[n]
                    if n in W_POOL_NS:
                        # v = dist*(kn/2pi); round+subtract on GpSimd via magic
                        v = sb.tile([P, M_CHUNK], f32, tag="v", bufs=3)
                        nc.vector.tensor_scalar(v[:], dist_t[:], kn / TWO_PI,
                                                None, mybir.AluOpType.mult)
                        vb = sb.tile([P, M_CHUNK], f32, tag="vb", bufs=2)
                        nc.gpsimd.tensor_tensor(vb[:], v[:], magic_t[:],
                                                mybir.AluOpType.add)
                        qf = sb.tile([P, M_CHUNK], f32, tag="qf", bufs=2)
                        nc.gpsimd.tensor_tensor(qf[:], vb[:], magic_t[:],
                                                mybir.AluOpType.subtract)
                        w = sb.tile([P, M_CHUNK], f32, tag="w", bufs=3)
                        nc.gpsimd.tensor_tensor(w[:], v[:], qf[:],
                                                mybir.AluOpType.subtract)
                        sin_scale = TWO_PI
                    else:
                        q = sb.tile([P, M_CHUNK], mybir.dt.int32, tag="q", bufs=3)
                        if n in Q_ACT_NS:
                            _raw_activation(nc.scalar, q[:], dist_t[:],
                                            mybir.ActivationFunctionType.Identity,
                                            scale=kn / TWO_PI)
                        else:
                            nc.vector.tensor_scalar(q[:], dist_t[:], kn / TWO_PI,
                                                    None, mybir.AluOpType.mult)
                        w = sb.tile([P, M_CHUNK], f32, tag="w", bufs=3)
                        nc.vector.scalar_tensor_tensor(w[:], q[:], -TWO_PI / kn,
                                                       dist_t[:],
                                                       mybir.AluOpType.mult,
                                                       mybir.AluOpType.add)
                        sin_scale = kn
                    sin_t = sb.tile([P, M_CHUNK], f32, tag="sin", bufs=4)
                    nc.scalar.activation(sin_t[:], w[:],
                                         mybir.ActivationFunctionType.Sin,
                                         scale=sin_scale)
                    exp_t = sb.tile([P, M_CHUNK], f32, tag="exp", bufs=6)
                    if n in POOL_NS:
                        nc.gpsimd.tensor_tensor(exp_t[:], sin_t[:], recip[:],
                                                mybir.AluOpType.mult)
                    else:
                        nc.vector.tensor_tensor(exp_t[:], sin_t[:], recip[:],
                                                mybir.AluOpType.mult)
                    nc.sync.dma_start(exp_d[n].ap()[rows, cols], exp_t[:])

    nc.compile()
    return nc


def _get_nc():
    if "nc" not in _RUNTIME:
        _RUNTIME["nc"] = _build_program()
    return _RUNTIME["nc"]


def make_in_maps(source, target):
    """Host-side packing: per-core input dicts."""
    source = np.asarray(source, np.float32).reshape(N_FULL, 3)
    target = np.asarray(target, np.float32).reshape(M_FULL, 3)
    trs = {}
    for c in range(3):
        t2 = np.empty((2, M_FULL), np.float32)
        t2[0] = target[:, c]
        t2[1] = -1.0
        trs[f"tr{c}"] = t2
    in_maps = []
    for core in range(N_CORES):
        s = source[core * N_LOC:(core + 1) * N_LOC]
        m = dict(trs)
        for c in range(3):
            s2 = np.empty((2, N_LOC), np.float32)
            s2[0] = 1.0
            s2[1] = s[:, c]
            m[f"sl{c}"] = s2
        in_maps.append(m)
    return in_maps


def assemble(results):
    """Gather per-core output dicts into the full reference-shaped tuple."""
    dist = np.concatenate([r["dist_o"] for r in results], 0)[None]
    mask = np.concatenate([r["mask_o"] for r in results], 0)[None].astype(bool)
    direction = np.stack(
        [np.concatenate([r[f"dir{c}_o"] for r in results], 0) for c in range(3)],
        axis=-1)[None]
    expansion = np.stack(
        [np.concatenate([r[f"exp{n}_o"] for r in results], 0) for n in range(FEAT)],
        axis=-1)[None]
    return dist, direction, mask, expansion


def kernel(source, target):
    from concourse import bass2jax

    nc = _get_nc()
    in_maps = make_in_maps(source, target)
    results = bass2jax.run_bass_via_pjrt(nc, in_maps, n_cores=N_CORES)
    return assemble(results)


# revision 18
# speedup vs baseline: 144.1855x; 1.0781x over previous
"""Trainium2 Bass kernel for DeepDFT pairwise expansion.

Computes, for source probes s[N=4096,3] and target atoms t[M=2048,3]:
  dist[n,m]      = |t[m] - s[n]|
  direction      = (t[m]-s[n]) / (dist + 1e-11)
  mask           = dist < 4.0
  expansion[...] = sin(k_n * dist) / (BOHR*dist + 1e-11),  k_n = n*pi*BOHR/4

Sharding: source axis N split across 8 NeuronCores (512 rows each);
targets replicated. Each core emits 25 f32 [512,2048] planes + u8 mask.

Per [128,512] block:
  PE   : diff_c = t_c - s_c  (K=2 matmul: lhsT=[ones; s_c], rhs=[t_c; -1])
  ACT  : squares of diffs (from PSUM), sqrt, 20x sin (range-reduced args)
  POOL : dist^2 adds + a slice of the expansion multiplies
  DVE  : mask (dist^2<16), d, 1/d, direction, magic-round range reduction
Range reduction: q = round(arg/2pi) via float32 magic-number add;
w = dist - q*(2pi/k_n); sin via ACT with scale=k_n (arg lands in [-pi,pi]).
"""
from contextlib import ExitStack

import numpy as np

N_FULL = 4096
M_FULL = 2048
FEAT = 20
N_CORES = 8
N_LOC = N_FULL // N_CORES          # 512
P = 128                            # SBUF partitions
N_TILES = N_LOC // P               # 4 row-tiles per core
M_CHUNK = 1024
M_CHUNKS = M_FULL // M_CHUNK

CUT = 4.0
EPS = 1e-11
BOHR = 0.5291772105638411
TWO_PI = 2.0 * np.pi
MAGIC = float(1.5 * 2.0**23)

K_N = [float(n * np.pi * BOHR / CUT) for n in range(1, FEAT + 1)]
# feature indices whose final multiply runs on GpSimd (rest on DVE)
POOL_NS = set(range(6, FEAT))

_RUNTIME = {}
REPS = 1        # body repetitions (measurement only; overwrites same outputs)
RECIP_MODE = "newton"   # "newton" (ACT recip + 1 NR step) or "dve" (8-cyc divide)
PS_BUFS = 3             # PSUM diff tile slots
SCRATCH_OUT = False     # timing-only: route stores to a small scratch tensor
Q_ACT_NS = {0, 1, 2, 3}  # features whose round() runs on ACT (Identity -> i32)
SQ_DVE = False          # squares on DVE instead of ACT (model probe)
MASK_F32 = False        # mask as f32 plane (model probe: +3% DMA)
W_POOL_NS = set()       # features whose round+w run on GpSimd via magic add


def _raw_activation(eng, out_ap, in_ap, func, scale=1.0, bias=0.0):
    import concourse.mybir as mybir
    ins = [eng.lower_ap(in_ap),
           mybir.ImmediateValue(dtype=mybir.dt.float32, value=float(bias)),
           mybir.ImmediateValue(dtype=mybir.dt.float32, value=float(scale)),
           mybir.ImmediateValue(dtype=mybir.dt.float32, value=0.0)]
    return eng.add_instruction(mybir.InstActivation(
        name=eng.bass.get_next_instruction_name(), func=func,
        ins=ins, outs=[eng.lower_ap(out_ap)]))


def _build_program():
    import concourse.tile as tile
    import concourse.mybir as mybir
    from concourse import bacc

    global M_CHUNKS
    M_CHUNKS = M_FULL // M_CHUNK

    nc = bacc.Bacc("TRN2", target_bir_lowering=False, debug=False)
    f32 = mybir.dt.float32

    tr_d = [nc.dram_tensor(f"tr{c}", [2, M_FULL], f32, kind="ExternalInput")
            for c in range(3)]
    sl_d = [nc.dram_tensor(f"sl{c}", [2, N_LOC], f32, kind="ExternalInput")
            for c in range(3)]

    okind = "Internal" if SCRATCH_OUT else "ExternalOutput"
    dist_d = nc.dram_tensor("dist_o", [N_LOC, M_FULL], f32, kind=okind)
    mask_d = nc.dram_tensor("mask_o", [N_LOC, M_FULL],
                            f32 if MASK_F32 else mybir.dt.uint8,
                            kind=okind)
    dir_d = [nc.dram_tensor(f"dir{c}_o", [N_LOC, M_FULL], f32, kind=okind)
             for c in range(3)]
    exp_d = [nc.dram_tensor(f"exp{n}_o", [N_LOC, M_FULL], f32, kind=okind)
             for n in range(FEAT)]
    token_d = None
    if SCRATCH_OUT:
        token_d = nc.dram_tensor("token_o", [2, 64], f32, kind="ExternalOutput")

    def store(dst_ap, tile_ap, plane, blk):
        nc.sync.dma_start(dst_ap, tile_ap)

    with tile.TileContext(nc) as tc, ExitStack() as ctx:
        const = ctx.enter_context(tc.tile_pool(name="const", bufs=1))
        sb = ctx.enter_context(tc.tile_pool(name="sb", bufs=2))
        ps = ctx.enter_context(tc.tile_pool(name="ps", bufs=PS_BUFS, space="PSUM"))

        tr = [const.tile([2, M_FULL], f32, name=f"tr{c}_t") for c in range(3)]
        sl = [const.tile([2, N_LOC], f32, name=f"sl{c}_t") for c in range(3)]
        for c in range(3):
            nc.sync.dma_start(tr[c][:], tr_d[c].ap())
            nc.sync.dma_start(sl[c][:], sl_d[c].ap())
        magic_t = None
        if W_POOL_NS:
            magic_t = const.tile([P, M_CHUNK], f32, name="magic_t")
            nc.vector.memset(magic_t[:], MAGIC)

        for _rep in range(REPS):
          for r in range(N_TILES):
            rows = slice(r * P, (r + 1) * P)
            for j in range(M_CHUNKS):
                cols = slice(j * M_CHUNK, (j + 1) * M_CHUNK)

                diff = []
                for c in range(3):
                    dps = ps.tile([P, M_CHUNK], f32, tag="diff")
                    for h in range(M_CHUNK // 512):
                        nc.tensor.matmul(
                            dps[:, h * 512:(h + 1) * 512], sl[c][:, rows],
                            tr[c][:, j * M_CHUNK + h * 512:
                                   j * M_CHUNK + (h + 1) * 512],
                            start=True, stop=True)
                    diff.append(dps)

                sq = []
                for c in range(3):
                    s = sb.tile([P, M_CHUNK], f32, tag="sq", bufs=4)
                    if SQ_DVE:
                        nc.vector.tensor_tensor(s[:], diff[c][:], diff[c][:],
                                                mybir.AluOpType.mult)
                    else:
                        nc.scalar.square(s[:], diff[c][:])
                    sq.append(s)

                s01 = sb.tile([P, M_CHUNK], f32, tag="s01")
                nc.gpsimd.tensor_tensor(s01[:], sq[0][:], sq[1][:],
                                        mybir.AluOpType.add)
                dist2 = sb.tile([P, M_CHUNK], f32, tag="dist2")
                nc.gpsimd.tensor_tensor(dist2[:], s01[:], sq[2][:],
                                        mybir.AluOpType.add)

                mask_t = sb.tile([P, M_CHUNK],
                                 f32 if MASK_F32 else mybir.dt.uint8, tag="mask", bufs=3)
                nc.vector.tensor_scalar(mask_t[:], dist2[:], float(CUT * CUT), None,
                                        mybir.AluOpType.is_lt)
                store(mask_d.ap()[rows, cols], mask_t[:], 0, r * M_CHUNKS + j)

                dist_t = sb.tile([P, M_CHUNK], f32, tag="dist")
                nc.scalar.sqrt(dist_t[:], dist2[:])
                store(dist_d.ap()[rows, cols], dist_t[:], 1, r * M_CHUNKS + j)

                if RECIP_MODE == "act":
                    recip = sb.tile([P, M_CHUNK], f32, tag="recip")
                    _raw_activation(nc.scalar, recip[:], dist_t[:],
                                    mybir.ActivationFunctionType.Reciprocal,
                                    scale=BOHR, bias=EPS)
                elif RECIP_MODE == "newton":
                    r0 = sb.tile([P, M_CHUNK], f32, tag="r0")
                    _raw_activation(nc.scalar, r0[:], dist_t[:],
                                    mybir.ActivationFunctionType.Reciprocal,
                                    scale=BOHR, bias=EPS)
                    nt = sb.tile([P, M_CHUNK], f32, tag="nt")
                    nc.vector.scalar_tensor_tensor(nt[:], dist_t[:], BOHR, r0[:],
                                                   mybir.AluOpType.mult,
                                                   mybir.AluOpType.mult)
                    nu = sb.tile([P, M_CHUNK], f32, tag="nu")
                    nc.vector.tensor_scalar(nu[:], nt[:], -1.0, 2.0,
                                            mybir.AluOpType.mult,
                                            mybir.AluOpType.add)
                    recip = sb.tile([P, M_CHUNK], f32, tag="recip")
                    nc.vector.tensor_tensor(recip[:], nu[:], r0[:],
                                            mybir.AluOpType.mult)
                else:
                    d_t = sb.tile([P, M_CHUNK], f32, tag="d")
                    nc.vector.tensor_scalar(d_t[:], dist_t[:], BOHR, EPS,
                                            mybir.AluOpType.mult,
                                            mybir.AluOpType.add)
                    recip = sb.tile([P, M_CHUNK], f32, tag="recip")
                    nc.vector.reciprocal(recip[:], d_t[:])

                for c in range(3):
                    dir_t = sb.tile([P, M_CHUNK], f32, tag="dir", bufs=6)
                    nc.vector.scalar_tensor_tensor(dir_t[:], diff[c][:], BOHR,
                                                   recip[:], mybir.AluOpType.mult,
                                                   mybir.AluOpType.mult)
                    store(dir_d[c].ap()[rows, cols], dir_t[:], 2 + c, r * M_CHUNKS + j)

                for n in range(FEAT):
                    kn = K_N[n]
                    if n in W_POOL_NS:
                        # v = dist*(kn/2pi); round+subtract on GpSimd via magic
                        v = sb.tile([P, M_CHUNK], f32, tag="v", bufs=3)
                        nc.vector.tensor_scalar(v[:], dist_t[:], kn / TWO_PI,
                                                None, mybir.AluOpType.mult)
                        vb = sb.tile([P, M_CHUNK], f32, tag="vb", bufs=2)
                        nc.gpsimd.tensor_tensor(vb[:], v[:], magic_t[:],
                                                mybir.AluOpType.add)
                        qf = sb.tile([P, M_CHUNK], f32, tag="qf", bufs=2)
                        nc.gpsimd.tensor_tensor(qf[:], vb[:], magic_t[:],
                                                mybir.AluOpType.subtract)
                        w = sb.tile([P, M_CHUNK], f32, tag="w", bufs=3)
                        nc.gpsimd.tensor_tensor(w[:], v[:], qf[:],
                                                mybir.AluOpType.subtract)
                        sin_scale = TWO_PI
                    else:
                        q = sb.tile([P, M_CHUNK], mybir.dt.int32, tag="q", bufs=3)
                        if n in Q_ACT_NS:
                            _raw_activation(nc.scalar, q[:], dist_t[:],
                                            mybir.ActivationFunctionType.Identity,
                                            scale=kn / TWO_PI)
                        else:
                            nc.vector.tensor_scalar(q[:], dist_t[:], kn / TWO_PI,
                                                    None, mybir.AluOpType.mult)
                        w = sb.tile([P, M_CHUNK], f32, tag="w", bufs=3)
                        nc.vector.scalar_tensor_tensor(w[:], q[:], -TWO_PI / kn,
                                                       dist_t[:],
                                                       mybir.AluOpType.mult,
                                                       mybir.AluOpType.add)
                        sin_scale = kn
                    sin_t = sb.tile([P, M_CHUNK], f32, tag="sin", bufs=4)
                    nc.scalar.activation(sin_t[:], w[:],
                                         mybir.ActivationFunctionType.Sin,
                                         scale=sin_scale)
                    exp_t = sb.tile([P, M_CHUNK], f32, tag="exp", bufs=6)
                    if n in POOL_NS:
                        nc.gpsimd.tensor_tensor(exp_t[:], sin_t[:], recip[:],
                                                mybir.AluOpType.mult)
                    else:
                        nc.vector.tensor_tensor(exp_t[:], sin_t[:], recip[:],
                                                mybir.AluOpType.mult)
                    store(exp_d[n].ap()[rows, cols], exp_t[:], 5 + n, r * M_CHUNKS + j)

        if SCRATCH_OUT:
            nc.sync.dma_start(token_d.ap(), tr[0][:, :64])

    nc.compile()
    return nc


def _get_nc():
    if "nc" not in _RUNTIME:
        _RUNTIME["nc"] = _build_program()
    return _RUNTIME["nc"]


def make_in_maps(source, target):
    """Host-side packing: per-core input dicts."""
    source = np.asarray(source, np.float32).reshape(N_FULL, 3)
    target = np.asarray(target, np.float32).reshape(M_FULL, 3)
    trs = {}
    for c in range(3):
        t2 = np.empty((2, M_FULL), np.float32)
        t2[0] = target[:, c]
        t2[1] = -1.0
        trs[f"tr{c}"] = t2
    in_maps = []
    for core in range(N_CORES):
        s = source[core * N_LOC:(core + 1) * N_LOC]
        m = dict(trs)
        for c in range(3):
            s2 = np.empty((2, N_LOC), np.float32)
            s2[0] = 1.0
            s2[1] = s[:, c]
            m[f"sl{c}"] = s2
        in_maps.append(m)
    return in_maps


def assemble(results):
    """Gather per-core output dicts into the full reference-shaped tuple."""
    dist = np.concatenate([r["dist_o"] for r in results], 0)[None]
    mask = np.concatenate([r["mask_o"] for r in results], 0)[None].astype(bool)
    direction = np.stack(
        [np.concatenate([r[f"dir{c}_o"] for r in results], 0) for c in range(3)],
        axis=-1)[None]
    expansion = np.stack(
        [np.concatenate([r[f"exp{n}_o"] for r in results], 0) for n in range(FEAT)],
        axis=-1)[None]
    return dist, direction, mask, expansion


def kernel(source, target):
    from concourse import bass2jax

    nc = _get_nc()
    in_maps = make_in_maps(source, target)
    results = bass2jax.run_bass_via_pjrt(nc, in_maps, n_cores=N_CORES)
    return assemble(results)
